# revision 1
# baseline (speedup 1.0000x reference)
"""Hausdorff distance kernel for Trainium2 (8 NeuronCores, Bass/Tile).

Pipeline:
  host   : binary masks -> edge point sets (raster order, truncated to 32768)
           capped separable EDT -> exact per-source 1-NN upper bounds
           morton-sorted source chunks (128 pts) + AABB candidate lists
           greedy LPT packing of chunks onto 8 cores (slot-aligned tile counts)
  device : per chunk: d^2 = phi(src) . psi(cand) via TensorE matmul (K=5 lift),
           VectorE min-reduce per source, per-chunk [128] mins -> DRAM
  host   : max-merge per directed pair, HD = sqrt(max(h_ab, h_ba)) per batch

Distances are exact: all coordinates are small integers, every product/sum
stays below 2^24 so fp32 arithmetic is exact end to end.
"""

import os
import numpy as np

GRID = 128          # D == H == W of the voxel grid
K_MAX = 32768       # reference truncates edge sets to this many points
CH = 128            # source points per chunk (= PSUM partitions)
TILE_N = 512        # matmul free-dim per instruction (= one PSUM bank)
EDT_CAP = 24        # per-axis cap of the host EDT used for pruning bounds
N_CORES = 8

_prog_cache = {}


# ----------------------------------------------------------------- host side

def _edge_points(mask):
    """mask [D,H,W] bool -> edge points [N,3] float32, raster order, <=K_MAX.

    Edge voxel = not in mask but with a set voxel in its 3x3x3 neighborhood,
    matching the reference conv + (neigh>0) & ~mask definition.
    """
    D, H, W = mask.shape
    p = np.pad(mask, 1)
    neigh = np.zeros_like(mask)
    for dz in range(3):
        for dy in range(3):
            for dx in range(3):
                neigh |= p[dz:dz + D, dy:dy + H, dx:dx + W]
    edge = neigh & ~mask
    pts = np.argwhere(edge)
    return pts[:K_MAX].astype(np.float32)


def _capped_edt_sq(tgt_pts, qry_pts, cap=EDT_CAP):
    """Exact min squared distance from each query point to the target set,
    computed by capped separable brute-force EDT on a cropped grid.
    Entries are +inf where the nearest target is farther than `cap` on some
    axis; finite entries are exact. Always a valid upper bound."""
    allpts = np.concatenate([tgt_pts, qry_pts], 0).astype(np.int64)
    lo = allpts.min(0)
    hi = allpts.max(0) + 1
    shape = tuple((hi - lo).tolist())
    INF = np.float32(3e18)
    g = np.full(shape, INF, np.float32)
    ti = tgt_pts.astype(np.int64) - lo
    g[ti[:, 0], ti[:, 1], ti[:, 2]] = 0.0
    for ax in range(3):
        res = np.full_like(g, INF)
        n = g.shape[ax]
        for s in range(-cap, cap + 1):
            if abs(s) >= n:
                continue
            src = [slice(None)] * 3
            dst = [slice(None)] * 3
            if s >= 0:
                src[ax] = slice(0, n - s)
                dst[ax] = slice(s, None)
            else:
                src[ax] = slice(-s, None)
                dst[ax] = slice(0, n + s)
            np.minimum(res[tuple(dst)], g[tuple(src)] + np.float32(s * s),
                       out=res[tuple(dst)])
        g = res
    qi = qry_pts.astype(np.int64) - lo
    out = g[qi[:, 0], qi[:, 1], qi[:, 2]].astype(np.float64)
    out[out > 1e18] = np.inf
    return out


def _morton(pts):
    x = pts.astype(np.int64)
    code = np.zeros(len(pts), np.int64)
    for b in range(7):
        for d in range(3):
            code |= ((x[:, d] >> b) & 1) << (3 * b + d)
    return code


DIAG2_MAX = 800     # cut chunks when the cumulative AABB diagonal^2 exceeds this
SUB = 16            # sub-chunk granularity for candidate bounds
COL_Q = 128         # candidate-column quantum (matmul free-dim granularity)


def _chunk_bounds(S):
    """Greedy cut points: grow each chunk up to CH points while its AABB
    diagonal^2 stays under DIAG2_MAX (morton order keeps runs compact)."""
    bounds = []
    i = 0
    N = len(S)
    while i < N:
        seg = S[i:min(i + CH, N)]
        lo = np.minimum.accumulate(seg, 0)
        hi = np.maximum.accumulate(seg, 0)
        diag2 = ((hi - lo) ** 2).sum(1)
        k = int(np.searchsorted(diag2, DIAG2_MAX, side="right"))
        k = max(min(k, len(seg)), min(32, len(seg)))
        bounds.append((i, i + k))
        i += k
    return bounds


def _build_chunks(S, T, ub2):
    """Split morton-sorted S into compact chunks; per chunk collect the
    candidate targets that can be some source's nearest neighbor (AABB lower
    bound vs per-source exact upper bound, at sub-chunk granularity)."""
    order = np.argsort(_morton(S), kind="stable")
    S = S[order]
    ub2 = ub2[order]
    chunks = []
    for c0, c1 in _chunk_bounds(S):
        s = S[c0:c1]
        u = ub2[c0:c1]
        mask = np.zeros(len(T), bool)
        for s0 in range(0, len(s), SUB):
            ss = s[s0:s0 + SUB]
            ub = u[s0:s0 + SUB].max()
            if not np.isfinite(ub):
                mask[:] = True
                break
            lo = ss.min(0)
            hi = ss.max(0)
            lb2 = (np.maximum(np.maximum(lo - T, T - hi), 0.0) ** 2).sum(1)
            mask |= lb2 <= ub
        cand = T[mask]
        if len(s) < CH:
            s = np.concatenate([s, np.repeat(s[:1], CH - len(s), 0)], 0)
        chunks.append((s, cand))
    return chunks


K_LIFT = 7  # d^2 as a K=7 inner product; every factor is an integer that is
            # exactly representable in bf16 (<=2^8 significand), and every
            # product/partial sum is an integer < 2^24, so fp32 PSUM
            # accumulation reproduces the fp32 reference bit-exactly.


def _phi(s):  # [N,3] -> [7,N] lifted sources (stationary operand), bf16-exact
    n2 = (s * s).sum(1).astype(np.int64)
    return np.stack([
        s[:, 0], s[:, 1], s[:, 2],
        (n2 >> 8).astype(np.float32), (n2 & 255).astype(np.float32),
        np.ones(len(s), np.float32), np.ones(len(s), np.float32),
    ]).astype(np.float32)


def _psi(t):  # [N,3] -> [7,N] lifted targets (moving operand), bf16-exact
    n2 = (t * t).sum(1).astype(np.int64)
    return np.stack([
        -2.0 * t[:, 0], -2.0 * t[:, 1], -2.0 * t[:, 2],
        np.full(len(t), 256.0, np.float32), np.ones(len(t), np.float32),
        ((n2 >> 8) << 8).astype(np.float32), (n2 & 255).astype(np.float32),
    ]).astype(np.float32)


# --------------------------------------------------------------- device side

def _build_program(NCH, slot_cols):
    """slot_cols[c]: candidate columns of chunk-slot c (multiple of COL_Q).
    Per slot: matmuls in <=TILE_N pieces, min-reduces over <=4-bank psum
    groups, final per-slot reduce into allbest[:, c]."""
    from concourse import bacc, tile
    import concourse.mybir as mybir

    f32 = mybir.dt.float32
    bf16 = mybir.dt.bfloat16
    GCOL = 2 * TILE_N  # psum columns (2 banks) per reduce instruction
    TOT = sum(slot_cols)

    nc = bacc.Bacc(None, target_bir_lowering=False)
    lhsT_d = nc.dram_tensor("lhsT", [K_LIFT, NCH * CH], bf16, kind="ExternalInput")
    rhs_d = nc.dram_tensor("rhs", [K_LIFT, TOT], bf16, kind="ExternalInput")
    out_d = nc.dram_tensor("out", [CH, NCH], f32, kind="ExternalOutput")

    with tile.TileContext(nc) as tc:
        with tc.tile_pool(name="w", bufs=1) as wpool, \
             tc.tile_pool(name="rhs", bufs=4) as rpool, \
             tc.tile_pool(name="red", bufs=4) as redpool, \
             tc.tile_pool(name="fin", bufs=1) as finpool, \
             tc.tile_pool(name="psum", bufs=4, space="PSUM") as ppool:
            lhsT = wpool.tile([K_LIFT, NCH * CH], bf16)
            nc.sync.dma_start(lhsT[:], lhsT_d[:])
            allbest = finpool.tile([CH, NCH], f32)
            off = 0
            for c in range(NCH):
                cols = slot_cols[c]
                ngroups = (cols + GCOL - 1) // GCOL
                rtile = rpool.tile([K_LIFT, cols], bf16, tag="rhs")
                nc.sync.dma_start(rtile[:], rhs_d[:, off:off + cols])
                bc = redpool.tile([CH, ngroups], f32, tag="bc")
                for g in range(ngroups):
                    gw = min(GCOL, cols - g * GCOL)
                    ps = ppool.tile([CH, GCOL], f32, tag="ps")
                    q = 0
                    while q < gw:
                        w = min(TILE_N, gw - q)
                        nc.tensor.matmul(
                            ps[:, q:q + w],
                            lhsT[:, c * CH:(c + 1) * CH],
                            rtile[:, g * GCOL + q:g * GCOL + q + w],
                            start=True, stop=True,
                        )
                        q += w
                    nc.vector.tensor_reduce(
                        bc[:, g:g + 1], ps[:, :gw],
                        axis=mybir.AxisListType.X, op=mybir.AluOpType.min,
                    )
                nc.vector.tensor_reduce(
                    allbest[:, c:c + 1], bc[:],
                    axis=mybir.AxisListType.X, op=mybir.AluOpType.min,
                )
                off += cols
            nc.sync.dma_start(out_d[:], allbest[:])
    nc.compile()
    return nc


# ------------------------------------------------------------------- kernel

def kernel(inputs, targets):
    inputs = np.asarray(inputs)
    targets = np.asarray(targets)
    B = inputs.shape[0]
    out = np.zeros(B, np.float32)

    # one work item per (batch, direction)
    items = []           # (dir_id, src_chunk[CH,3], cand[M,3])
    n_dirs = 0
    dir_of_batch = {}    # batch -> (dir_ab, dir_ba)
    for b in range(B):
        a = (inputs[b] > 0).any(0)
        t = (targets[b] > 0).any(0)
        pa = _edge_points(a)
        pt = _edge_points(t)
        if len(pa) == 0 or len(pt) == 0:
            out[b] = np.inf
            continue
        ub_ab = _capped_edt_sq(pt, pa)
        ub_ba = _capped_edt_sq(pa, pt)
        d_ab, d_ba = n_dirs, n_dirs + 1
        n_dirs += 2
        dir_of_batch[b] = (d_ab, d_ba)
        for s, c in _build_chunks(pa, pt, ub_ab):
            items.append((d_ab, s, c))
        for s, c in _build_chunks(pt, pa, ub_ba):
            items.append((d_ba, s, c))

    if not items:
        return out

    # greedy LPT packing onto 8 cores; descending column count keeps per-slot
    # column counts aligned across cores (the SPMD program is shared)
    cols_of = lambda it: ((len(it[2]) + COL_Q - 1) // COL_Q) * COL_Q
    order = sorted(range(len(items)), key=lambda i: -cols_of(items[i]))
    per_core = [[] for _ in range(N_CORES)]
    load = [0] * N_CORES
    for i in order:
        k = load.index(min(load))
        per_core[k].append(items[i])
        load[k] += cols_of(items[i])

    NCH = max(1, max(len(c) for c in per_core))
    slot_cols = []
    for c in range(NCH):
        w = COL_Q
        for k in range(N_CORES):
            if c < len(per_core[k]):
                w = max(w, cols_of(per_core[k][c]))
        slot_cols.append(w)
    TOT = sum(slot_cols)

    import ml_dtypes
    bf16_np = ml_dtypes.bfloat16

    in_maps = []
    for k in range(N_CORES):
        lhsT_np = np.zeros((K_LIFT, NCH * CH), np.float32)
        rhs_np = np.zeros((K_LIFT, TOT), np.float32)
        off = 0
        for c in range(NCH):
            it = None
            if c < len(per_core[k]):
                it = per_core[k][c]
            elif per_core[k]:
                it = per_core[k][0]   # replicated filler; host ignores slot
            if it is not None:
                _, s, cand = it
                lhsT_np[:, c * CH:(c + 1) * CH] = _phi(s)
                need = slot_cols[c]
                idx = np.arange(need) % len(cand)
                rhs_np[:, off:off + need] = _psi(cand[idx])
            off += slot_cols[c]
        in_maps.append({"lhsT": lhsT_np.astype(bf16_np),
                        "rhs": rhs_np.astype(bf16_np)})

    key = (NCH, tuple(slot_cols))
    if key not in _prog_cache:
        _prog_cache[key] = _build_program(NCH, slot_cols)
    nc = _prog_cache[key]

    from concourse.bass_utils import run_bass_kernel_spmd
    trace = bool(os.environ.get("HD_TRACE"))
    try:
        res = run_bass_kernel_spmd(nc, in_maps, list(range(N_CORES)), trace=trace)
    except Exception:
        if not trace:
            raise
        res = run_bass_kernel_spmd(nc, in_maps, list(range(N_CORES)), trace=False)
    if trace and res.exec_time_ns is not None:
        print(f"HW exec time: {res.exec_time_ns} ns")

    # max-merge per direction on host
    h2 = np.zeros(n_dirs, np.float64)
    for k in range(N_CORES):
        o = np.asarray(res.results[k]["out"])  # [CH, NCH]
        for c, (d, _, _) in enumerate(per_core[k]):
            h2[d] = max(h2[d], float(o[:, c].max()))

    for b, (d_ab, d_ba) in dir_of_batch.items():
        out[b] = np.sqrt(np.float32(max(h2[d_ab], h2[d_ba])))
    return out



# revision 5
# speedup vs baseline: 1.1032x; 1.1032x over previous
"""Hausdorff distance kernel for Trainium2 (8 NeuronCores, Bass/Tile).

Pipeline:
  host   : binary masks -> edge point sets (raster order, truncated to 32768)
           capped separable EDT -> exact per-source 1-NN upper bounds
           morton-sorted source chunks (<=64 pts) + AABB candidate lists
           per chunk, candidates split into 2 halves -> the two halves are
           computed by two matmuls into partition groups [0:64) and [64:128)
           of the same PSUM columns, halving the drained width at no extra
           PE cost; greedy LPT packing of chunk-slots onto 8 cores
  device : per slot: d^2 = phi(src) . psi(cand) via TensorE matmul (K=7
           lift), one VectorE min-reduce of [128, W] per slot -> out[:, c]
  host   : min over the two 64-row groups, max-merge per directed pair,
           HD = sqrt(max(h_ab, h_ba)) per batch

Distances are exact: all coordinates are small integers, every product/sum
stays below 2^24 so fp32 arithmetic is exact end to end.
"""

import os
import numpy as np

GRID = 128          # D == H == W of the voxel grid
K_MAX = 32768       # reference truncates edge sets to this many points
SRC_CH = 64         # source points per chunk (half the PSUM partitions)
KSPLIT = 2          # candidate halves -> partition groups per chunk
CH = 128            # PSUM partitions
TILE_N = 512        # matmul free-dim per instruction (= one PSUM bank)
PSUM_W = 2048       # psum tile width (4 banks)
EDT_CAP = 24        # per-axis cap of the host EDT used for pruning bounds
N_CORES = 8
DMA_SPAN = 14336    # rhs columns per input DMA (elem < 64KB per partition)

DIAG2_MAX = 800     # cut chunks when the cumulative AABB diagonal^2 exceeds this
SUB = 8             # sub-chunk granularity for candidate bounds
COL_Q = 64          # candidate-column quantum per half

_prog_cache = {}


# ----------------------------------------------------------------- host side

def _edge_points(mask):
    """mask [D,H,W] bool -> edge points [N,3] float32, raster order, <=K_MAX.

    Edge voxel = not in mask but with a set voxel in its 3x3x3 neighborhood,
    matching the reference conv + (neigh>0) & ~mask definition.
    """
    D, H, W = mask.shape
    p = np.pad(mask, 1)
    neigh = np.zeros_like(mask)
    for dz in range(3):
        for dy in range(3):
            for dx in range(3):
                neigh |= p[dz:dz + D, dy:dy + H, dx:dx + W]
    edge = neigh & ~mask
    pts = np.argwhere(edge)
    return pts[:K_MAX].astype(np.float32)


def _capped_edt_sq(tgt_pts, qry_pts, cap=EDT_CAP):
    """Exact min squared distance from each query point to the target set,
    computed by capped separable brute-force EDT on a cropped grid.
    Entries are +inf where the nearest target is farther than `cap` on some
    axis; finite entries are exact. Always a valid upper bound."""
    allpts = np.concatenate([tgt_pts, qry_pts], 0).astype(np.int64)
    lo = allpts.min(0)
    hi = allpts.max(0) + 1
    shape = tuple((hi - lo).tolist())
    INF = np.float32(3e18)
    g = np.full(shape, INF, np.float32)
    ti = tgt_pts.astype(np.int64) - lo
    g[ti[:, 0], ti[:, 1], ti[:, 2]] = 0.0
    for ax in range(3):
        res = np.full_like(g, INF)
        n = g.shape[ax]
        for s in range(-cap, cap + 1):
            if abs(s) >= n:
                continue
            src = [slice(None)] * 3
            dst = [slice(None)] * 3
            if s >= 0:
                src[ax] = slice(0, n - s)
                dst[ax] = slice(s, None)
            else:
                src[ax] = slice(-s, None)
                dst[ax] = slice(0, n + s)
            np.minimum(res[tuple(dst)], g[tuple(src)] + np.float32(s * s),
                       out=res[tuple(dst)])
        g = res
    qi = qry_pts.astype(np.int64) - lo
    out = g[qi[:, 0], qi[:, 1], qi[:, 2]].astype(np.float64)
    out[out > 1e18] = np.inf
    return out


def _morton(pts):
    x = pts.astype(np.int64)
    code = np.zeros(len(pts), np.int64)
    for b in range(7):
        for d in range(3):
            code |= ((x[:, d] >> b) & 1) << (3 * b + d)
    return code


def _chunk_bounds(S):
    """Greedy cut points: grow each chunk up to SRC_CH points while its AABB
    diagonal^2 stays under DIAG2_MAX (morton order keeps runs compact)."""
    bounds = []
    i = 0
    N = len(S)
    while i < N:
        seg = S[i:min(i + SRC_CH, N)]
        lo = np.minimum.accumulate(seg, 0)
        hi = np.maximum.accumulate(seg, 0)
        diag2 = ((hi - lo) ** 2).sum(1)
        k = int(np.searchsorted(diag2, DIAG2_MAX, side="right"))
        k = max(min(k, len(seg)), min(16, len(seg)))
        bounds.append((i, i + k))
        i += k
    return bounds


def _build_chunks(S, T, ub2):
    """Split morton-sorted S into compact chunks; per chunk collect the
    candidate targets that can be some source's nearest neighbor (AABB lower
    bound vs per-source exact upper bound, at sub-chunk granularity)."""
    order = np.argsort(_morton(S), kind="stable")
    S = S[order]
    ub2 = ub2[order]
    chunks = []
    for c0, c1 in _chunk_bounds(S):
        s = S[c0:c1]
        u = ub2[c0:c1]
        mask = np.zeros(len(T), bool)
        for s0 in range(0, len(s), SUB):
            ss = s[s0:s0 + SUB]
            ub = u[s0:s0 + SUB].max()
            if not np.isfinite(ub):
                mask[:] = True
                break
            lo = ss.min(0)
            hi = ss.max(0)
            lb2 = (np.maximum(np.maximum(lo - T, T - hi), 0.0) ** 2).sum(1)
            mask |= lb2 <= ub
        cand = T[mask]
        if len(s) < SRC_CH:
            s = np.concatenate([s, np.repeat(s[:1], SRC_CH - len(s), 0)], 0)
        chunks.append((s, cand))
    return chunks


K_LIFT = 7  # d^2 as a K=7 inner product; every factor is an integer that is
            # exactly representable in bf16 (<=2^8 significand), and every
            # product/partial sum is an integer < 2^24, so fp32 PSUM
            # accumulation reproduces the fp32 reference bit-exactly.


def _phi(s):  # [N,3] -> [7,N] lifted sources (stationary operand), bf16-exact
    n2 = (s * s).sum(1).astype(np.int64)
    return np.stack([
        s[:, 0], s[:, 1], s[:, 2],
        (n2 >> 8).astype(np.float32), (n2 & 255).astype(np.float32),
        np.ones(len(s), np.float32), np.ones(len(s), np.float32),
    ]).astype(np.float32)


def _psi(t):  # [N,3] -> [7,N] lifted targets (moving operand), bf16-exact
    n2 = (t * t).sum(1).astype(np.int64)
    return np.stack([
        -2.0 * t[:, 0], -2.0 * t[:, 1], -2.0 * t[:, 2],
        np.full(len(t), 256.0, np.float32), np.ones(len(t), np.float32),
        ((n2 >> 8) << 8).astype(np.float32), (n2 & 255).astype(np.float32),
    ]).astype(np.float32)


# --------------------------------------------------------------- device side

def _build_program(NCH, slot_ws):
    """slot_ws[c]: drained width (per candidate half) of chunk-slot c.
    Per slot: two matmuls (candidate halves) into partition groups [0:64)
    and [64:128) of one [128, W] psum region, one min-reduce -> out[:, c].
    The rhs stays resident in SBUF, loaded by a few large DMAs up front."""
    from concourse import bacc, tile
    import concourse.mybir as mybir

    f32 = mybir.dt.float32
    bf16 = mybir.dt.bfloat16
    TOT = 2 * sum(slot_ws)

    nc = bacc.Bacc(None, target_bir_lowering=False)
    lhsT_d = nc.dram_tensor("lhsT", [K_LIFT, NCH * SRC_CH], bf16,
                            kind="ExternalInput")
    rhs_d = nc.dram_tensor("rhs", [K_LIFT, TOT], bf16, kind="ExternalInput")
    out_d = nc.dram_tensor("out", [CH, NCH], f32, kind="ExternalOutput")

    with tile.TileContext(nc) as tc:
        with tc.tile_pool(name="w", bufs=1) as wpool, \
             tc.tile_pool(name="fin", bufs=1) as finpool, \
             tc.tile_pool(name="psum", bufs=2, space="PSUM") as ppool:
            lhsT = wpool.tile([K_LIFT, NCH * SRC_CH], bf16)
            nc.sync.dma_start(lhsT[:], lhsT_d[:])
            rhs = wpool.tile([K_LIFT, TOT], bf16)
            span = 0
            while span < TOT:
                sw = min(DMA_SPAN, TOT - span)
                nc.sync.dma_start(rhs[:, span:span + sw],
                                  rhs_d[:, span:span + sw])
                span += sw
            allbest = finpool.tile([CH, NCH], f32)
            off = 0
            for c in range(NCH):
                W = slot_ws[c]
                ps = ppool.tile([CH, PSUM_W], f32, tag="ps", name="ps")
                lhs_ap = lhsT[:, c * SRC_CH:(c + 1) * SRC_CH]
                for g in range(KSPLIT):
                    rowo = g * SRC_CH
                    q = 0
                    while q < W:
                        w = min(TILE_N, W - q)
                        nc.tensor.matmul(
                            ps[rowo:rowo + SRC_CH, q:q + w],
                            lhs_ap,
                            rhs[:, off + g * W + q:off + g * W + q + w],
                            start=True, stop=True,
                        )
                        q += w
                nc.vector.tensor_reduce(
                    allbest[:, c:c + 1], ps[:, :W],
                    axis=mybir.AxisListType.X, op=mybir.AluOpType.min,
                )
                off += 2 * W
            nc.sync.dma_start(out_d[:], allbest[:])
    nc.compile()
    return nc


# ------------------------------------------------------------------- kernel

def kernel(inputs, targets):
    inputs = np.asarray(inputs)
    targets = np.asarray(targets)
    B = inputs.shape[0]
    out = np.zeros(B, np.float32)

    # one work item per (batch, direction)
    items = []           # (dir_id, src_chunk[SRC_CH,3], cand[M,3])
    n_dirs = 0
    dir_of_batch = {}    # batch -> (dir_ab, dir_ba)
    for b in range(B):
        a = (inputs[b] > 0).any(0)
        t = (targets[b] > 0).any(0)
        pa = _edge_points(a)
        pt = _edge_points(t)
        if len(pa) == 0 or len(pt) == 0:
            out[b] = np.inf
            continue
        ub_ab = _capped_edt_sq(pt, pa)
        ub_ba = _capped_edt_sq(pa, pt)
        d_ab, d_ba = n_dirs, n_dirs + 1
        n_dirs += 2
        dir_of_batch[b] = (d_ab, d_ba)
        for s, c in _build_chunks(pa, pt, ub_ab):
            items.append((d_ab, s, c))
        for s, c in _build_chunks(pt, pa, ub_ba):
            items.append((d_ba, s, c))

    if not items:
        return out

    # drained width per item: candidate halves, quantized
    def w_of(it):
        m = len(it[2])
        return max(COL_Q, (((m + 1) // 2 + COL_Q - 1) // COL_Q) * COL_Q)

    # greedy LPT packing onto 8 cores by predicted slot cost
    cost_of = lambda it: w_of(it) * 1.042 + 125.0
    order = sorted(range(len(items)), key=lambda i: -cost_of(items[i]))
    per_core = [[] for _ in range(N_CORES)]
    load = [0.0] * N_CORES
    for i in order:
        k = load.index(min(load))
        per_core[k].append(items[i])
        load[k] += cost_of(items[i])

    NCH = max(1, max(len(c) for c in per_core))
    slot_ws = []
    for c in range(NCH):
        w = COL_Q
        for k in range(N_CORES):
            if c < len(per_core[k]):
                w = max(w, w_of(per_core[k][c]))
        slot_ws.append(w)
    TOT = 2 * sum(slot_ws)

    import ml_dtypes
    bf16_np = ml_dtypes.bfloat16

    in_maps = []
    for k in range(N_CORES):
        lhsT_np = np.zeros((K_LIFT, NCH * SRC_CH), np.float32)
        rhs_np = np.zeros((K_LIFT, TOT), np.float32)
        off = 0
        for c in range(NCH):
            it = None
            if c < len(per_core[k]):
                it = per_core[k][c]
            elif per_core[k]:
                it = per_core[k][0]   # replicated filler; host ignores slot
            W = slot_ws[c]
            if it is not None:
                _, s, cand = it
                lhsT_np[:, c * SRC_CH:(c + 1) * SRC_CH] = _phi(s)
                idx = np.arange(2 * W) % len(cand)
                rhs_np[:, off:off + 2 * W] = _psi(cand[idx])
            off += 2 * W
        in_maps.append({"lhsT": lhsT_np.astype(bf16_np),
                        "rhs": rhs_np.astype(bf16_np)})

    key = (NCH, tuple(slot_ws))
    if key not in _prog_cache:
        _prog_cache[key] = _build_program(NCH, slot_ws)
    nc = _prog_cache[key]

    from concourse.bass_utils import run_bass_kernel_spmd
    trace = bool(os.environ.get("HD_TRACE"))
    try:
        res = run_bass_kernel_spmd(nc, in_maps, list(range(N_CORES)), trace=trace)
    except Exception:
        if not trace:
            raise
        res = run_bass_kernel_spmd(nc, in_maps, list(range(N_CORES)), trace=False)
    if trace and res.exec_time_ns is not None:
        print(f"HW exec time: {res.exec_time_ns} ns")

    # min over the two partition groups, then max-merge per direction
    h2 = np.zeros(n_dirs, np.float64)
    for k in range(N_CORES):
        o = np.asarray(res.results[k]["out"])  # [CH, NCH]
        for c, (d, _, _) in enumerate(per_core[k]):
            m = np.minimum(o[:SRC_CH, c], o[SRC_CH:, c]).max()
            h2[d] = max(h2[d], float(m))

    for b, (d_ab, d_ba) in dir_of_batch.items():
        out[b] = np.sqrt(np.float32(max(h2[d_ab], h2[d_ba])))
    return out


# revision 6
# speedup vs baseline: 3.4045x; 3.0862x over previous
"""Hausdorff distance kernel for Trainium2 (8 NeuronCores, Bass/Tile).

Pipeline:
  host   : binary masks -> edge point sets (raster order, truncated to 32768)
           capped separable EDT -> exact per-source 1-NN upper bounds
           morton-sorted 16-point source chunks + AABB candidate pruning at
           4-point sub-chunk granularity
           each chunk's candidate list is split into "bands" of <=96; eight
           bands (from any chunks) share one matmul slot: band b occupies
           lift-rows [7b,7b+7) and output partitions [16b,16b+16) of a
           zero-block-diagonal [56,128] stationary, so one 96-column matmul
           evaluates 8x96 candidate pairs -- PE and PSUM cost are paid per
           drained column, not per candidate
  device : per slot one K=56 matmul into a shared PSUM tile (20 slots per
           [128,2048] tile, 5 per bank); one batched VectorE min-reduce per
           tile -> out[:, slot]
  host   : per chunk min over its bands' 16-row groups, max-merge per
           directed pair, HD = sqrt(max(h_ab, h_ba)) per batch

Distances are exact: all coordinates are small integers, every product/sum
stays below 2^24 so fp32 arithmetic is exact end to end.
"""

import os
import numpy as np

GRID = 128          # D == H == W of the voxel grid
K_MAX = 32768       # reference truncates edge sets to this many points
SRC_CH = 16         # source points per chunk (= partition group height)
NBAND = 8           # bands per matmul slot (8 x 16 partitions = 128)
BAND_W = 96         # candidate columns per band/slot
SLOTS_PER_BANKROW = 5   # floor(512 / 96) slots per PSUM bank
TILE_SLOTS = 4 * SLOTS_PER_BANKROW  # slots per [128,2048] psum tile
PSUM_W = 2048
EDT_CAP = 24        # per-axis cap of the host EDT used for pruning bounds
N_CORES = 8
K_LIFT = 7
K_ROWS = NBAND * K_LIFT  # 56

DIAG2_MAX = 100     # cut chunks when the cumulative AABB diagonal^2 exceeds this
SUB = 4             # sub-chunk granularity for candidate bounds

_prog_cache = {}


# ----------------------------------------------------------------- host side

def _edge_points(mask):
    """mask [D,H,W] bool -> edge points [N,3] float32, raster order, <=K_MAX.

    Edge voxel = not in mask but with a set voxel in its 3x3x3 neighborhood,
    matching the reference conv + (neigh>0) & ~mask definition.
    """
    D, H, W = mask.shape
    p = np.pad(mask, 1)
    neigh = np.zeros_like(mask)
    for dz in range(3):
        for dy in range(3):
            for dx in range(3):
                neigh |= p[dz:dz + D, dy:dy + H, dx:dx + W]
    edge = neigh & ~mask
    pts = np.argwhere(edge)
    return pts[:K_MAX].astype(np.float32)


def _capped_edt_sq(tgt_pts, qry_pts, cap=EDT_CAP):
    """Exact min squared distance from each query point to the target set,
    computed by capped separable brute-force EDT on a cropped grid.
    Entries are +inf where the nearest target is farther than `cap` on some
    axis; finite entries are exact. Always a valid upper bound."""
    allpts = np.concatenate([tgt_pts, qry_pts], 0).astype(np.int64)
    lo = allpts.min(0)
    hi = allpts.max(0) + 1
    shape = tuple((hi - lo).tolist())
    INF = np.float32(3e18)
    g = np.full(shape, INF, np.float32)
    ti = tgt_pts.astype(np.int64) - lo
    g[ti[:, 0], ti[:, 1], ti[:, 2]] = 0.0
    for ax in range(3):
        res = np.full_like(g, INF)
        n = g.shape[ax]
        for s in range(-cap, cap + 1):
            if abs(s) >= n:
                continue
            src = [slice(None)] * 3
            dst = [slice(None)] * 3
            if s >= 0:
                src[ax] = slice(0, n - s)
                dst[ax] = slice(s, None)
            else:
                src[ax] = slice(-s, None)
                dst[ax] = slice(0, n + s)
            np.minimum(res[tuple(dst)], g[tuple(src)] + np.float32(s * s),
                       out=res[tuple(dst)])
        g = res
    qi = qry_pts.astype(np.int64) - lo
    out = g[qi[:, 0], qi[:, 1], qi[:, 2]].astype(np.float64)
    out[out > 1e18] = np.inf
    return out


def _morton(pts):
    x = pts.astype(np.int64)
    code = np.zeros(len(pts), np.int64)
    for b in range(7):
        for d in range(3):
            code |= ((x[:, d] >> b) & 1) << (3 * b + d)
    return code


def _chunk_bounds(S):
    """Greedy cut points: grow each chunk up to SRC_CH points while its AABB
    diagonal^2 stays under DIAG2_MAX (morton order keeps runs compact)."""
    bounds = []
    i = 0
    N = len(S)
    while i < N:
        seg = S[i:min(i + SRC_CH, N)]
        lo = np.minimum.accumulate(seg, 0)
        hi = np.maximum.accumulate(seg, 0)
        diag2 = ((hi - lo) ** 2).sum(1)
        k = int(np.searchsorted(diag2, DIAG2_MAX, side="right"))
        k = max(min(k, len(seg)), min(SUB, len(seg)))
        bounds.append((i, i + k))
        i += k
    return bounds


def _build_chunks(S, T, ub2):
    """Split morton-sorted S into compact chunks; per chunk collect the
    candidate targets that can be some source's nearest neighbor (AABB lower
    bound vs per-source exact upper bound, at sub-chunk granularity)."""
    order = np.argsort(_morton(S), kind="stable")
    S = S[order]
    ub2 = ub2[order]
    chunks = []
    for c0, c1 in _chunk_bounds(S):
        s = S[c0:c1]
        u = ub2[c0:c1]
        mask = np.zeros(len(T), bool)
        for s0 in range(0, len(s), SUB):
            ss = s[s0:s0 + SUB]
            ub = u[s0:s0 + SUB].max()
            if not np.isfinite(ub):
                mask[:] = True
                break
            lo = ss.min(0)
            hi = ss.max(0)
            lb2 = (np.maximum(np.maximum(lo - T, T - hi), 0.0) ** 2).sum(1)
            mask |= lb2 <= ub
        cand = T[mask]
        if len(s) < SRC_CH:
            s = np.concatenate([s, np.repeat(s[:1], SRC_CH - len(s), 0)], 0)
        chunks.append((s, cand))
    return chunks


def _phi(s):  # [N,3] -> [7,N] lifted sources (stationary operand), bf16-exact
    n2 = (s * s).sum(1).astype(np.int64)
    return np.stack([
        s[:, 0], s[:, 1], s[:, 2],
        (n2 >> 8).astype(np.float32), (n2 & 255).astype(np.float32),
        np.ones(len(s), np.float32), np.ones(len(s), np.float32),
    ]).astype(np.float32)


def _psi(t):  # [N,3] -> [7,N] lifted targets (moving operand), bf16-exact
    n2 = (t * t).sum(1).astype(np.int64)
    return np.stack([
        -2.0 * t[:, 0], -2.0 * t[:, 1], -2.0 * t[:, 2],
        np.full(len(t), 256.0, np.float32), np.ones(len(t), np.float32),
        ((n2 >> 8) << 8).astype(np.float32), (n2 & 255).astype(np.float32),
    ]).astype(np.float32)


# --------------------------------------------------------------- device side

def _build_program(NSLOT):
    """NSLOT uniform slots. Per slot: one K=56 matmul of [128, 96] into a
    shared psum tile (20 slots per tile, 5 per 512-col bank with a 32-col
    gap); per tile one batched min-reduce [128, 4, 5, 96] -> 20 out columns.
    lhsT and rhs stay resident in SBUF, streamed in per-tile spans."""
    from concourse import bacc, tile
    import concourse.mybir as mybir

    f32 = mybir.dt.float32
    bf16 = mybir.dt.bfloat16

    ntiles = (NSLOT + TILE_SLOTS - 1) // TILE_SLOTS
    assert NSLOT % TILE_SLOTS == 0, "caller pads slot count to tile multiple"

    nc = bacc.Bacc(None, target_bir_lowering=False)
    lhsT_d = nc.dram_tensor("lhsT", [K_ROWS, NSLOT * 128], bf16,
                            kind="ExternalInput")
    rhs_d = nc.dram_tensor("rhs", [K_ROWS, NSLOT * BAND_W], bf16,
                           kind="ExternalInput")
    out_d = nc.dram_tensor("out", [128, NSLOT], f32, kind="ExternalOutput")

    with tile.TileContext(nc) as tc:
        with tc.tile_pool(name="w", bufs=1) as wpool, \
             tc.tile_pool(name="fin", bufs=1) as finpool, \
             tc.tile_pool(name="psum", bufs=2, space="PSUM") as ppool:
            lhsT = wpool.tile([K_ROWS, NSLOT * 128], bf16)
            rhs = wpool.tile([K_ROWS, NSLOT * BAND_W], bf16)
            for t in range(ntiles):
                s0 = t * TILE_SLOTS
                s1 = min(NSLOT, s0 + TILE_SLOTS)
                nc.sync.dma_start(lhsT[:, s0 * 128:s1 * 128],
                                  lhsT_d[:, s0 * 128:s1 * 128])
                nc.sync.dma_start(rhs[:, s0 * BAND_W:s1 * BAND_W],
                                  rhs_d[:, s0 * BAND_W:s1 * BAND_W])
            allbest = finpool.tile([128, NSLOT], f32)
            for t in range(ntiles):
                ps = ppool.tile([128, PSUM_W], f32, tag="ps", name="ps")
                for j in range(TILE_SLOTS):
                    g = t * TILE_SLOTS + j
                    bank, r = divmod(j, SLOTS_PER_BANKROW)
                    off = bank * 512 + r * BAND_W
                    nc.tensor.matmul(
                        ps[:, off:off + BAND_W],
                        lhsT[:, g * 128:(g + 1) * 128],
                        rhs[:, g * BAND_W:(g + 1) * BAND_W],
                        start=True, stop=True,
                    )
                src4 = (ps[:, :]
                        .rearrange("p (b r) -> p b r", b=4)[:, :, :SLOTS_PER_BANKROW * BAND_W]
                        .rearrange("p b (s w) -> p b s w", s=SLOTS_PER_BANKROW))
                dst3 = (allbest[:, t * TILE_SLOTS:(t + 1) * TILE_SLOTS]
                        .rearrange("p (b s) -> p b s", b=4))
                nc.vector.tensor_reduce(
                    dst3, src4,
                    axis=mybir.AxisListType.X, op=mybir.AluOpType.min,
                )
            nc.sync.dma_start(out_d[:], allbest[:])
    nc.compile()
    return nc


# ------------------------------------------------------------------- kernel

def kernel(inputs, targets):
    inputs = np.asarray(inputs)
    targets = np.asarray(targets)
    B = inputs.shape[0]
    out = np.zeros(B, np.float32)

    # one work item per (batch, direction); chunks of 16 sources
    chunks = []          # (dir_id, src[16,3], cand[M,3])
    n_dirs = 0
    dir_of_batch = {}    # batch -> (dir_ab, dir_ba)
    for b in range(B):
        a = (inputs[b] > 0).any(0)
        t = (targets[b] > 0).any(0)
        pa = _edge_points(a)
        pt = _edge_points(t)
        if len(pa) == 0 or len(pt) == 0:
            out[b] = np.inf
            continue
        ub_ab = _capped_edt_sq(pt, pa)
        ub_ba = _capped_edt_sq(pa, pt)
        d_ab, d_ba = n_dirs, n_dirs + 1
        n_dirs += 2
        dir_of_batch[b] = (d_ab, d_ba)
        for s, c in _build_chunks(pa, pt, ub_ab):
            chunks.append((d_ab, s, c))
        for s, c in _build_chunks(pt, pa, ub_ba):
            chunks.append((d_ba, s, c))

    if not chunks:
        return out

    # chunk -> bands of <=BAND_W candidates (cyclically padded to BAND_W)
    nband_of = lambda ci: max(1, -(-len(chunks[ci][2]) // BAND_W))

    # LPT on band count across cores
    order = sorted(range(len(chunks)), key=lambda i: -nband_of(i))
    core_chunks = [[] for _ in range(N_CORES)]
    load = [0] * N_CORES
    for i in order:
        k = load.index(min(load))
        core_chunks[k].append(i)
        load[k] += nband_of(i)

    max_bands = max(load)
    NSLOT = -(-max_bands // NBAND)
    NSLOT = -(-NSLOT // TILE_SLOTS) * TILE_SLOTS   # pad to whole psum tiles

    import ml_dtypes
    bf16_np = ml_dtypes.bfloat16

    in_maps = []
    band_maps = []       # per core: list of (chunk_idx, cand_group) per band
    for k in range(N_CORES):
        bands = []
        for ci in core_chunks[k]:
            _, s, cand = chunks[ci]
            nb = nband_of(ci)
            for g in range(nb):
                bands.append((ci, g))
        while len(bands) < NSLOT * NBAND:
            bands.append(bands[0] if bands else (core_chunks[0][0] if core_chunks[0] else 0, 0))
        band_maps.append(bands)

        lhsT_np = np.zeros((K_ROWS, NSLOT * 128), np.float32)
        rhs_np = np.zeros((K_ROWS, NSLOT * BAND_W), np.float32)
        for i, (ci, g) in enumerate(bands):
            slot, b = divmod(i, NBAND)
            _, s, cand = chunks[ci]
            lhsT_np[7 * b:7 * b + 7,
                    slot * 128 + 16 * b:slot * 128 + 16 * b + 16] = _phi(s)
            idx = (g * BAND_W + np.arange(BAND_W)) % len(cand)
            rhs_np[7 * b:7 * b + 7,
                   slot * BAND_W:(slot + 1) * BAND_W] = _psi(cand[idx])
        in_maps.append({"lhsT": lhsT_np.astype(bf16_np),
                        "rhs": rhs_np.astype(bf16_np)})

    if NSLOT not in _prog_cache:
        _prog_cache[NSLOT] = _build_program(NSLOT)
    nc = _prog_cache[NSLOT]

    from concourse.bass_utils import run_bass_kernel_spmd
    trace = bool(os.environ.get("HD_TRACE"))
    try:
        res = run_bass_kernel_spmd(nc, in_maps, list(range(N_CORES)), trace=trace)
    except Exception:
        if not trace:
            raise
        res = run_bass_kernel_spmd(nc, in_maps, list(range(N_CORES)), trace=False)
    if trace and res.exec_time_ns is not None:
        print(f"HW exec time: {res.exec_time_ns} ns")

    # host merge: per band the 16-row group of its slot column holds that
    # band's per-source mins; min across a chunk's bands, then max per dir
    chunk_min = {}
    for k in range(N_CORES):
        o = np.asarray(res.results[k]["out"])  # [128, NSLOT]
        used = len(core_chunks[k]) and sum(nband_of(ci) for ci in core_chunks[k])
        for i, (ci, g) in enumerate(band_maps[k][:used]):
            slot, b = divmod(i, NBAND)
            v = o[16 * b:16 * b + 16, slot]
            if ci in chunk_min:
                chunk_min[ci] = np.minimum(chunk_min[ci], v)
            else:
                chunk_min[ci] = v.copy()

    h2 = np.zeros(n_dirs, np.float64)
    for ci, v in chunk_min.items():
        d = chunks[ci][0]
        h2[d] = max(h2[d], float(v.max()))

    for b, (d_ab, d_ba) in dir_of_batch.items():
        out[b] = np.sqrt(np.float32(max(h2[d_ab], h2[d_ba])))
    return out


# revision 7
# speedup vs baseline: 3.4819x; 1.0227x over previous
"""Hausdorff distance kernel for Trainium2 (8 NeuronCores, Bass/Tile).

Pipeline:
  host   : binary masks -> edge point sets (raster order, truncated to 32768)
           capped separable EDT -> exact per-source 1-NN upper bounds
           morton-sorted 16-point source chunks + AABB candidate pruning at
           4-point sub-chunk granularity
           each chunk's candidate list is split into "bands" of <=96; eight
           bands (from any chunks) share one matmul slot: band b occupies
           lift-rows [7b,7b+7) and output partitions [16b,16b+16) of a
           zero-block-diagonal [56,128] stationary, so one 96-column matmul
           evaluates 8x96 candidate pairs -- PE and PSUM cost are paid per
           drained column, not per candidate
  device : per slot one K=56 matmul into a shared PSUM tile (20 slots per
           [128,2048] tile, 5 per bank); one batched VectorE min-reduce per
           tile -> out[:, slot]
  host   : per chunk min over its bands' 16-row groups, max-merge per
           directed pair, HD = sqrt(max(h_ab, h_ba)) per batch

Distances are exact: all coordinates are small integers, every product/sum
stays below 2^24 so fp32 arithmetic is exact end to end.
"""

import os
import numpy as np

GRID = 128          # D == H == W of the voxel grid
K_MAX = 32768       # reference truncates edge sets to this many points
SRC_CH = 16         # source points per chunk (= partition group height)
NBAND = 8           # bands per matmul slot (8 x 16 partitions = 128)
BAND_W = 64         # candidate columns per band/slot
SLOTS_PER_BANKROW = 8   # 512 / 64 slots per PSUM bank, gap-free
TILE_SLOTS = 4 * SLOTS_PER_BANKROW  # slots per [128,2048] psum tile
PSUM_W = 2048
EDT_CAP = 24        # per-axis cap of the host EDT used for pruning bounds
N_CORES = 8
K_LIFT = 7
K_ROWS = NBAND * K_LIFT  # 56

DIAG2_MAX = 100     # cut chunks when the cumulative AABB diagonal^2 exceeds this
SUB = 4             # sub-chunk granularity for candidate bounds

_prog_cache = {}


# ----------------------------------------------------------------- host side

def _edge_points(mask):
    """mask [D,H,W] bool -> edge points [N,3] float32, raster order, <=K_MAX.

    Edge voxel = not in mask but with a set voxel in its 3x3x3 neighborhood,
    matching the reference conv + (neigh>0) & ~mask definition.
    """
    D, H, W = mask.shape
    p = np.pad(mask, 1)
    neigh = np.zeros_like(mask)
    for dz in range(3):
        for dy in range(3):
            for dx in range(3):
                neigh |= p[dz:dz + D, dy:dy + H, dx:dx + W]
    edge = neigh & ~mask
    pts = np.argwhere(edge)
    return pts[:K_MAX].astype(np.float32)


def _capped_edt_sq(tgt_pts, qry_pts, cap=EDT_CAP):
    """Exact min squared distance from each query point to the target set,
    computed by capped separable brute-force EDT on a cropped grid.
    Entries are +inf where the nearest target is farther than `cap` on some
    axis; finite entries are exact. Always a valid upper bound."""
    allpts = np.concatenate([tgt_pts, qry_pts], 0).astype(np.int64)
    lo = allpts.min(0)
    hi = allpts.max(0) + 1
    shape = tuple((hi - lo).tolist())
    INF = np.float32(3e18)
    g = np.full(shape, INF, np.float32)
    ti = tgt_pts.astype(np.int64) - lo
    g[ti[:, 0], ti[:, 1], ti[:, 2]] = 0.0
    for ax in range(3):
        res = np.full_like(g, INF)
        n = g.shape[ax]
        for s in range(-cap, cap + 1):
            if abs(s) >= n:
                continue
            src = [slice(None)] * 3
            dst = [slice(None)] * 3
            if s >= 0:
                src[ax] = slice(0, n - s)
                dst[ax] = slice(s, None)
            else:
                src[ax] = slice(-s, None)
                dst[ax] = slice(0, n + s)
            np.minimum(res[tuple(dst)], g[tuple(src)] + np.float32(s * s),
                       out=res[tuple(dst)])
        g = res
    qi = qry_pts.astype(np.int64) - lo
    out = g[qi[:, 0], qi[:, 1], qi[:, 2]].astype(np.float64)
    out[out > 1e18] = np.inf
    return out


def _morton(pts):
    x = pts.astype(np.int64)
    code = np.zeros(len(pts), np.int64)
    for b in range(7):
        for d in range(3):
            code |= ((x[:, d] >> b) & 1) << (3 * b + d)
    return code


def _chunk_bounds(S):
    """Greedy cut points: grow each chunk up to SRC_CH points while its AABB
    diagonal^2 stays under DIAG2_MAX (morton order keeps runs compact)."""
    bounds = []
    i = 0
    N = len(S)
    while i < N:
        seg = S[i:min(i + SRC_CH, N)]
        lo = np.minimum.accumulate(seg, 0)
        hi = np.maximum.accumulate(seg, 0)
        diag2 = ((hi - lo) ** 2).sum(1)
        k = int(np.searchsorted(diag2, DIAG2_MAX, side="right"))
        k = max(min(k, len(seg)), min(SUB, len(seg)))
        bounds.append((i, i + k))
        i += k
    return bounds


def _build_chunks(S, T, ub2):
    """Split morton-sorted S into compact chunks; per chunk collect the
    candidate targets that can be some source's nearest neighbor (AABB lower
    bound vs per-source exact upper bound, at sub-chunk granularity)."""
    order = np.argsort(_morton(S), kind="stable")
    S = S[order]
    ub2 = ub2[order]
    chunks = []
    for c0, c1 in _chunk_bounds(S):
        s = S[c0:c1]
        u = ub2[c0:c1]
        mask = np.zeros(len(T), bool)
        for s0 in range(0, len(s), SUB):
            ss = s[s0:s0 + SUB]
            ub = u[s0:s0 + SUB].max()
            if not np.isfinite(ub):
                mask[:] = True
                break
            lo = ss.min(0)
            hi = ss.max(0)
            lb2 = (np.maximum(np.maximum(lo - T, T - hi), 0.0) ** 2).sum(1)
            mask |= lb2 <= ub
        cand = T[mask]
        if len(s) < SRC_CH:
            s = np.concatenate([s, np.repeat(s[:1], SRC_CH - len(s), 0)], 0)
        chunks.append((s, cand))
    return chunks


def _phi(s):  # [N,3] -> [7,N] lifted sources (stationary operand), bf16-exact
    n2 = (s * s).sum(1).astype(np.int64)
    return np.stack([
        s[:, 0], s[:, 1], s[:, 2],
        (n2 >> 8).astype(np.float32), (n2 & 255).astype(np.float32),
        np.ones(len(s), np.float32), np.ones(len(s), np.float32),
    ]).astype(np.float32)


def _psi(t):  # [N,3] -> [7,N] lifted targets (moving operand), bf16-exact
    n2 = (t * t).sum(1).astype(np.int64)
    return np.stack([
        -2.0 * t[:, 0], -2.0 * t[:, 1], -2.0 * t[:, 2],
        np.full(len(t), 256.0, np.float32), np.ones(len(t), np.float32),
        ((n2 >> 8) << 8).astype(np.float32), (n2 & 255).astype(np.float32),
    ]).astype(np.float32)


# --------------------------------------------------------------- device side

def _build_program(NSLOT):
    """NSLOT uniform slots. Per slot: one K=56 matmul of [128, 96] into a
    shared psum tile (20 slots per tile, 5 per 512-col bank with a 32-col
    gap); per tile one batched min-reduce [128, 4, 5, 96] -> 20 out columns.
    lhsT and rhs stay resident in SBUF, streamed in per-tile spans."""
    from concourse import bacc, tile
    import concourse.mybir as mybir

    f32 = mybir.dt.float32
    bf16 = mybir.dt.bfloat16

    ntiles = (NSLOT + TILE_SLOTS - 1) // TILE_SLOTS
    assert NSLOT % TILE_SLOTS == 0, "caller pads slot count to tile multiple"

    nc = bacc.Bacc(None, target_bir_lowering=False)
    lhsT_d = nc.dram_tensor("lhsT", [K_ROWS, NSLOT * 128], bf16,
                            kind="ExternalInput")
    rhs_d = nc.dram_tensor("rhs", [K_ROWS, NSLOT * BAND_W], bf16,
                           kind="ExternalInput")
    out_d = nc.dram_tensor("out", [128, NSLOT], f32, kind="ExternalOutput")

    with tile.TileContext(nc) as tc:
        with tc.tile_pool(name="w", bufs=1) as wpool, \
             tc.tile_pool(name="fin", bufs=1) as finpool, \
             tc.tile_pool(name="psum", bufs=2, space="PSUM") as ppool:
            lhsT = wpool.tile([K_ROWS, NSLOT * 128], bf16)
            rhs = wpool.tile([K_ROWS, NSLOT * BAND_W], bf16)
            for t in range(ntiles):
                s0 = t * TILE_SLOTS
                s1 = min(NSLOT, s0 + TILE_SLOTS)
                nc.sync.dma_start(lhsT[:, s0 * 128:s1 * 128],
                                  lhsT_d[:, s0 * 128:s1 * 128])
                nc.sync.dma_start(rhs[:, s0 * BAND_W:s1 * BAND_W],
                                  rhs_d[:, s0 * BAND_W:s1 * BAND_W])
            allbest = finpool.tile([128, NSLOT], f32)
            for t in range(ntiles):
                ps = ppool.tile([128, PSUM_W], f32, tag="ps", name="ps")
                for j in range(TILE_SLOTS):
                    g = t * TILE_SLOTS + j
                    bank, r = divmod(j, SLOTS_PER_BANKROW)
                    off = bank * 512 + r * BAND_W
                    nc.tensor.matmul(
                        ps[:, off:off + BAND_W],
                        lhsT[:, g * 128:(g + 1) * 128],
                        rhs[:, g * BAND_W:(g + 1) * BAND_W],
                        start=True, stop=True,
                    )
                src4 = (ps[:, :]
                        .rearrange("p (b r) -> p b r", b=4)[:, :, :SLOTS_PER_BANKROW * BAND_W]
                        .rearrange("p b (s w) -> p b s w", s=SLOTS_PER_BANKROW))
                dst3 = (allbest[:, t * TILE_SLOTS:(t + 1) * TILE_SLOTS]
                        .rearrange("p (b s) -> p b s", b=4))
                nc.vector.tensor_reduce(
                    dst3, src4,
                    axis=mybir.AxisListType.X, op=mybir.AluOpType.min,
                )
            nc.sync.dma_start(out_d[:], allbest[:])
    nc.compile()
    return nc


# ------------------------------------------------------------------- kernel

def kernel(inputs, targets):
    inputs = np.asarray(inputs)
    targets = np.asarray(targets)
    B = inputs.shape[0]
    out = np.zeros(B, np.float32)

    # one work item per (batch, direction); chunks of 16 sources
    chunks = []          # (dir_id, src[16,3], cand[M,3])
    n_dirs = 0
    dir_of_batch = {}    # batch -> (dir_ab, dir_ba)
    for b in range(B):
        a = (inputs[b] > 0).any(0)
        t = (targets[b] > 0).any(0)
        pa = _edge_points(a)
        pt = _edge_points(t)
        if len(pa) == 0 or len(pt) == 0:
            out[b] = np.inf
            continue
        ub_ab = _capped_edt_sq(pt, pa)
        ub_ba = _capped_edt_sq(pa, pt)
        d_ab, d_ba = n_dirs, n_dirs + 1
        n_dirs += 2
        dir_of_batch[b] = (d_ab, d_ba)
        for s, c in _build_chunks(pa, pt, ub_ab):
            chunks.append((d_ab, s, c))
        for s, c in _build_chunks(pt, pa, ub_ba):
            chunks.append((d_ba, s, c))

    if not chunks:
        return out

    # chunk -> bands of <=BAND_W candidates (cyclically padded to BAND_W)
    nband_of = lambda ci: max(1, -(-len(chunks[ci][2]) // BAND_W))

    # LPT on band count across cores
    order = sorted(range(len(chunks)), key=lambda i: -nband_of(i))
    core_chunks = [[] for _ in range(N_CORES)]
    load = [0] * N_CORES
    for i in order:
        k = load.index(min(load))
        core_chunks[k].append(i)
        load[k] += nband_of(i)

    max_bands = max(load)
    NSLOT = -(-max_bands // NBAND)
    NSLOT = -(-NSLOT // TILE_SLOTS) * TILE_SLOTS   # pad to whole psum tiles

    import ml_dtypes
    bf16_np = ml_dtypes.bfloat16

    in_maps = []
    band_maps = []       # per core: list of (chunk_idx, cand_group) per band
    for k in range(N_CORES):
        bands = []
        for ci in core_chunks[k]:
            _, s, cand = chunks[ci]
            nb = nband_of(ci)
            for g in range(nb):
                bands.append((ci, g))
        while len(bands) < NSLOT * NBAND:
            bands.append(bands[0] if bands else (core_chunks[0][0] if core_chunks[0] else 0, 0))
        band_maps.append(bands)

        lhsT_np = np.zeros((K_ROWS, NSLOT * 128), np.float32)
        rhs_np = np.zeros((K_ROWS, NSLOT * BAND_W), np.float32)
        for i, (ci, g) in enumerate(bands):
            slot, b = divmod(i, NBAND)
            _, s, cand = chunks[ci]
            lhsT_np[7 * b:7 * b + 7,
                    slot * 128 + 16 * b:slot * 128 + 16 * b + 16] = _phi(s)
            idx = (g * BAND_W + np.arange(BAND_W)) % len(cand)
            rhs_np[7 * b:7 * b + 7,
                   slot * BAND_W:(slot + 1) * BAND_W] = _psi(cand[idx])
        in_maps.append({"lhsT": lhsT_np.astype(bf16_np),
                        "rhs": rhs_np.astype(bf16_np)})

    if NSLOT not in _prog_cache:
        _prog_cache[NSLOT] = _build_program(NSLOT)
    nc = _prog_cache[NSLOT]

    from concourse.bass_utils import run_bass_kernel_spmd
    trace = bool(os.environ.get("HD_TRACE"))
    try:
        res = run_bass_kernel_spmd(nc, in_maps, list(range(N_CORES)), trace=trace)
    except Exception:
        if not trace:
            raise
        res = run_bass_kernel_spmd(nc, in_maps, list(range(N_CORES)), trace=False)
    if trace and res.exec_time_ns is not None:
        print(f"HW exec time: {res.exec_time_ns} ns")

    # host merge: per band the 16-row group of its slot column holds that
    # band's per-source mins; min across a chunk's bands, then max per dir
    chunk_min = {}
    for k in range(N_CORES):
        o = np.asarray(res.results[k]["out"])  # [128, NSLOT]
        used = len(core_chunks[k]) and sum(nband_of(ci) for ci in core_chunks[k])
        for i, (ci, g) in enumerate(band_maps[k][:used]):
            slot, b = divmod(i, NBAND)
            v = o[16 * b:16 * b + 16, slot]
            if ci in chunk_min:
                chunk_min[ci] = np.minimum(chunk_min[ci], v)
            else:
                chunk_min[ci] = v.copy()

    h2 = np.zeros(n_dirs, np.float64)
    for ci, v in chunk_min.items():
        d = chunks[ci][0]
        h2[d] = max(h2[d], float(v.max()))

    for b, (d_ab, d_ba) in dir_of_batch.items():
        out[b] = np.sqrt(np.float32(max(h2[d_ab], h2[d_ba])))
    return out
